# revision 5
# baseline (speedup 1.0000x reference)
"""Trainium2 Bass kernel for nn_Attention_3599182594919.

Multi-head attention, B=8 N=2048 C=384 H=6 D=64, data-parallel over batch
across 8 NeuronCores (one batch element per core, no collectives).

Per-core algorithm (layouts chosen so no on-chip transposes are needed and
every DVE op keeps all operands on the same partition window):
  host:  xT = x[b].T                                   [C, N]
         emt[k, q] = exp(-1e5*(mask[b][q,k] - min_k mask[b][q,:]))  bf16 [N, N]
           (softmax max-shift folded into the mask factor; exp(a+b)=exp(a)*exp(b))
  dev:   qkT = [Wq; Wk] @ xT                           [2C, N]  (partition = feature)
         v_aug = [x[b] @ Wv.T | 1]                     [N, H, D+1] bf16
         per head pair hp, q-tile qt (512 q):
           S^T[k, q] = kT.T @ qT           (PE f32, even head rows 0-63 /
                                            odd head rows 64-127, concurrent)
           e = exp(0.125 * S^T)            (ACT, 4-bank PSUM read, bf16 out)
           P = e * emt                     (DVE bf16 2x)
           [O^T; l] += [v|1].T @ P         (PE bf16, PSUM accum over k-tiles)
           ao_h = O^T[0:64] * (1/l)        (1/l broadcast to 64 partitions via
                                            a K=1 matmul from partition 64)
         y[tok, :] = sum_h ao_h.T @ pw_h + b            (PE, 6 K=64 matmuls)
"""

from contextlib import ExitStack

import numpy as np
import ml_dtypes

import concourse.bass as bass
import concourse.mybir as mybir
from concourse import bacc
from concourse.tile import TileContext
from concourse.bass_utils import run_bass_kernel_spmd

F32 = mybir.dt.float32
BF16 = mybir.dt.bfloat16

B, N, C, H = 8, 2048, 384, 6
D = C // H          # 64
QT = N // 512       # 4  q-tiles of 512
KT = N // 128       # 16 k-tiles of 128
NT = N // 128       # 16 token tiles

# set by test harness to capture timing
TRACE = False
LAST_RESULT = None

_NC_CACHE = None


def build_nc():
    nc = bacc.Bacc("TRN2", target_bir_lowering=False, debug=False)

    xT = nc.declare_dram_parameter("xT", [C, N], F32, isOutput=False)
    wqkT = nc.declare_dram_parameter("wqkT", [C, 2 * C], F32, isOutput=False)
    wvT = nc.declare_dram_parameter("wvT", [C, C], F32, isOutput=False)
    pwT = nc.declare_dram_parameter("pwT", [C, C], F32, isOutput=False)
    pb = nc.declare_dram_parameter("pb", [1, C], F32, isOutput=False)
    emt = nc.declare_dram_parameter("emt", [N, N], BF16, isOutput=False)
    out = nc.declare_dram_parameter("out", [N, C], F32, isOutput=True)

    emt_r = emt.ap().rearrange("(t p) q -> p t q", p=128)  # [128, KT, N]

    with TileContext(nc) as tc:
        with ExitStack() as ctx:
            consts = ctx.enter_context(tc.tile_pool(name="consts", bufs=1))
            wpool = ctx.enter_context(tc.tile_pool(name="weights", bufs=1))
            qkpool = ctx.enter_context(tc.tile_pool(name="qk", bufs=1))
            vpool = ctx.enter_context(tc.tile_pool(name="v", bufs=1))
            spair = ctx.enter_context(tc.tile_pool(name="spair", bufs=1, space="PSUM"))
            pvp = ctx.enter_context(tc.tile_pool(name="pvp", bufs=4, space="PSUM"))
            xt_ctx = ExitStack()
            xtpool = xt_ctx.enter_context(tc.tile_pool(name="xt", bufs=1))

            # ---- constants ----
            ones_row = consts.tile([128, 64], F32, tag="ones_row")
            nc.vector.memset(ones_row[:, :], 1.0)
            ones1 = consts.tile([1, 128], F32, tag="ones1")
            nc.vector.memset(ones1[:, :], 1.0)
            pb_sb = consts.tile([1, C], F32, tag="pbsb")
            nc.sync.dma_start(out=pb_sb[:, :], in_=pb[:, :])

            # ---- load x^T and weights ----
            xT_sb = []
            for i in range(3):
                t = xtpool.tile([128, N], F32, tag=f"xT{i}")
                nc.sync.dma_start(out=t[:, :], in_=xT[i * 128:(i + 1) * 128, :])
                xT_sb.append(t)
            wqkT_sb = []
            for i in range(3):
                t = wpool.tile([128, 2 * C], F32, tag=f"wqk{i}")
                nc.sync.dma_start(out=t[:, :], in_=wqkT[i * 128:(i + 1) * 128, :])
                wqkT_sb.append(t)
            wvT_sb = []
            for i in range(3):
                t = wpool.tile([128, C], F32, tag=f"wv{i}")
                nc.sync.dma_start(out=t[:, :], in_=wvT[i * 128:(i + 1) * 128, :])
                wvT_sb.append(t)
            # proj weights, one [64, C] tile per head (all at partition base 0)
            pw6_sb = []
            for h in range(H):
                t = wpool.tile([64, C], F32, tag=f"pw{h}")
                nc.sync.dma_start(out=t[:, :], in_=pwT[h * 64:(h + 1) * 64, :])
                pw6_sb.append(t)

            # ---- phase B: qkT = [Wq; Wk] @ xT  -> [768, N] (6 partition tiles) ----
            qkT_sb = []
            for f in range(6):
                qkT_sb.append(qkpool.tile([128, N], F32, name=f"qk{f}", tag=f"qk{f}"))
            for f in range(6):
                for j in range(QT):
                    ps = pvp.tile([128, 512], F32, tag="pv")
                    for kc in range(3):
                        nc.tensor.matmul(
                            ps[:, :],
                            wqkT_sb[kc][:, f * 128:(f + 1) * 128],
                            xT_sb[kc][:, j * 512:(j + 1) * 512],
                            start=(kc == 0),
                            stop=(kc == 2),
                        )
                    nc.any.tensor_copy(qkT_sb[f][:, j * 512:(j + 1) * 512], ps[:, :])

            # ---- phase C: v_aug [N, H, D+1] bf16, ones in last column ----
            vaug_sb = []
            for t_i in range(NT):
                vaug_sb.append(vpool.tile([128, H, D + 1], BF16, name=f"va{t_i}", tag=f"va{t_i}"))
            for t_i in range(NT):
                ps = pvp.tile([128, C], F32, tag="pv")
                for kc in range(3):
                    nc.tensor.matmul(
                        ps[:, :],
                        xT_sb[kc][:, t_i * 128:(t_i + 1) * 128],
                        wvT_sb[kc][:, :],
                        start=(kc == 0),
                        stop=(kc == 2),
                    )
                nc.vector.memset(vaug_sb[t_i][:, :, :], 1.0)
                nc.any.tensor_copy(
                    vaug_sb[t_i][:, :, 0:D],
                    ps[:, :].rearrange("p (h d) -> p h d", d=D),
                )

            # xT no longer needed; free its SBUF for the attention pools
            xt_ctx.close()
            emtp = ctx.enter_context(tc.tile_pool(name="emtp", bufs=3))
            epool = ctx.enter_context(tc.tile_pool(name="e", bufs=2))
            ppool = ctx.enter_context(tc.tile_pool(name="p", bufs=3))
            aopool = ctx.enter_context(tc.tile_pool(name="ao", bufs=8))
            lpool = ctx.enter_context(tc.tile_pool(name="l", bufs=2))
            ypool = ctx.enter_context(tc.tile_pool(name="y", bufs=2))

            # ---- phase D: attention per q-tile ----
            for qt in range(QT):
                em_tiles = []
                for kg in range(2):
                    em = emtp.tile([128, 8 * 512], BF16, tag="emt")
                    nc.sync.dma_start(
                        out=em[:, :],
                        in_=emt_r[:, kg * 8:(kg + 1) * 8, qt * 512:(qt + 1) * 512],
                    )
                    em_tiles.append(em)
                ao_tiles = [None] * H
                for hp in range(3):
                    pv_e = pvp.tile([128, 512], F32, tag="pv")
                    pv_o = pvp.tile([128, 512], F32, tag="pv")
                    for kg in range(8):
                        kta, ktb = 2 * kg, 2 * kg + 1
                        sp = spair.tile([128, 2048], F32, tag="spair")
                        # S^T matmuls: even head on partitions 0-63 (row grp 0-1),
                        # odd head on partitions 64-127 (row grp 2-3) -> concurrent
                        for idx, kt in ((0, kta), (1, ktb)):
                            nc.tensor.matmul(
                                sp[:, idx * 512:(idx + 1) * 512],
                                qkT_sb[3 + hp][0:64, kt * 128:(kt + 1) * 128],
                                qkT_sb[hp][0:64, qt * 512:(qt + 1) * 512],
                                start=True,
                                stop=True,
                            )
                            nc.tensor.matmul(
                                sp[:, 1024 + idx * 512:1024 + (idx + 1) * 512],
                                qkT_sb[3 + hp][64:128, kt * 128:(kt + 1) * 128],
                                qkT_sb[hp][64:128, qt * 512:(qt + 1) * 512],
                                start=True,
                                stop=True,
                            )
                        et = epool.tile([128, 2048], BF16, tag="e")
                        nc.scalar.activation(
                            et[:, :],
                            sp[:, :],
                            mybir.ActivationFunctionType.Exp,
                            scale=0.125,
                        )
                        pt = ppool.tile([128, 2048], BF16, tag="p")
                        emsl = em_tiles[kg // 4][:, (kg % 4) * 1024:(kg % 4 + 1) * 1024]
                        nc.vector.tensor_mul(pt[:, 0:1024], et[:, 0:1024], emsl)
                        nc.vector.tensor_mul(pt[:, 1024:2048], et[:, 1024:2048], emsl)
                        for idx, kt in ((0, kta), (1, ktb)):
                            nc.tensor.matmul(
                                pv_e[0:D + 1, :],
                                vaug_sb[kt][:, 2 * hp, :],
                                pt[:, idx * 512:(idx + 1) * 512],
                                start=(kt == 0),
                                stop=(kt == KT - 1),
                            )
                            nc.tensor.matmul(
                                pv_o[0:D + 1, :],
                                vaug_sb[kt][:, 2 * hp + 1, :],
                                pt[:, 1024 + idx * 512:1024 + (idx + 1) * 512],
                                start=(kt == 0),
                                stop=(kt == KT - 1),
                            )
                    # l lives on partition 64 (row D of the [65,512] PV result).
                    # Keep it there: copy both heads' l to SBUF partition 64,
                    # reciprocal there, then a K=1 matmul from partition 64
                    # broadcasts 1/l down to partitions 0-63.
                    lrow = lpool.tile([65, 1024], F32, tag="lrow")
                    nc.vector.tensor_copy(lrow[64:65, 0:512], pv_e[D:D + 1, :])
                    nc.vector.tensor_copy(lrow[64:65, 512:1024], pv_o[D:D + 1, :])
                    bc_e = pvp.tile([128, 512], F32, tag="pv")
                    bc_o = pvp.tile([128, 512], F32, tag="pv")
                    nc.tensor.matmul(
                        bc_e[0:64, :], ones_row[64:65, :], lrow[64:65, 0:512],
                        start=True, stop=True,
                    )
                    nc.tensor.matmul(
                        bc_o[0:64, :], ones_row[64:65, :], lrow[64:65, 512:1024],
                        start=True, stop=True,
                    )
                    bcl_e = lpool.tile([64, 512], F32, tag="bcle")
                    bcl_o = lpool.tile([64, 512], F32, tag="bclo")
                    nc.any.tensor_copy(bcl_e[:, :], bc_e[0:64, :])
                    nc.any.tensor_copy(bcl_o[:, :], bc_o[0:64, :])
                    bcs_e = lpool.tile([64, 512], F32, tag="bcse")
                    bcs_o = lpool.tile([64, 512], F32, tag="bcso")
                    nc.vector.reciprocal_approx_fast(out=bcs_e[:, :], in_=bcl_e[:, :])
                    nc.vector.reciprocal_approx_fast(out=bcs_o[:, :], in_=bcl_o[:, :])
                    ao_e = aopool.tile([64, 512], F32, tag="ao")
                    ao_o = aopool.tile([64, 512], F32, tag="ao")
                    nc.vector.tensor_mul(ao_e[:, :], pv_e[0:64, :], bcs_e[:, :])
                    nc.vector.tensor_mul(ao_o[:, :], pv_o[0:64, :], bcs_o[:, :])
                    ao_tiles[2 * hp] = ao_e
                    ao_tiles[2 * hp + 1] = ao_o

                # ---- phase E: y = sum_h ao_h.T @ pw_h + b for this q-tile ----
                for tt in range(4):
                    ps = pvp.tile([128, C], F32, tag="pv")
                    for h in range(H):
                        nc.tensor.matmul(
                            ps[:, :],
                            ao_tiles[h][:, tt * 128:(tt + 1) * 128],
                            pw6_sb[h][:, :],
                            start=(h == 0),
                            stop=False,
                        )
                    nc.tensor.matmul(
                        ps[:, :], ones1[:, :], pb_sb[:, :], start=False, stop=True
                    )
                    yt = ypool.tile([128, C], F32, tag="y")
                    nc.any.tensor_copy(yt[:, :], ps[:, :])
                    row = (qt * 4 + tt) * 128
                    nc.sync.dma_start(out=out[row:row + 128, :], in_=yt[:, :])

    nc.compile()
    return nc


def _get_nc():
    global _NC_CACHE
    if _NC_CACHE is None:
        _NC_CACHE = build_nc()
    return _NC_CACHE


def kernel(**inputs):
    x = np.asarray(inputs["x"], dtype=np.float32)
    mask = np.asarray(inputs["mask"], dtype=np.float32)
    qkv_w = np.asarray(inputs["qkv_w"], dtype=np.float32)
    proj_w = np.asarray(inputs["proj_w"], dtype=np.float32)
    proj_b = np.asarray(inputs["proj_b"], dtype=np.float32)

    nc = _get_nc()

    wqkT = np.ascontiguousarray(qkv_w[:2 * C].T)
    wvT = np.ascontiguousarray(qkv_w[2 * C:].T)
    pwT = np.ascontiguousarray(proj_w.T)
    pb = np.ascontiguousarray(proj_b.reshape(1, C))

    in_maps = []
    for b in range(B):
        xT = np.ascontiguousarray(x[b].T)
        mm = mask[b] - mask[b].min(axis=1, keepdims=True)
        emt = np.exp(-1e5 * mm).T.astype(ml_dtypes.bfloat16)
        in_maps.append(
            {
                "xT": xT,
                "wqkT": wqkT,
                "wvT": wvT,
                "pwT": pwT,
                "pb": pb,
                "emt": np.ascontiguousarray(emt),
            }
        )

    global LAST_RESULT
    res = run_bass_kernel_spmd(nc, in_maps, core_ids=list(range(B)), trace=TRACE)
    LAST_RESULT = res
    return np.stack([res.results[b]["out"] for b in range(B)]).astype(np.float32)


# revision 6
# speedup vs baseline: 3.2862x; 3.2862x over previous
"""Trainium2 Bass kernel for nn_Attention_3599182594919.

Multi-head attention, B=8 N=2048 C=384 H=6 D=64, data-parallel over batch
across 8 NeuronCores (one batch element per core, no collectives).

Per-core algorithm (layouts chosen so no on-chip transposes are needed and
every DVE op keeps all operands on the same partition window):
  host:  xT = x[b].T bf16                              [C, N]
         emt[k, q] = exp(-1e5*(mask[b][q,k] - min_k mask[b][q,:]))  bf16 [N, N]
           (softmax max-shift folded into the mask factor; exp(a+b)=exp(a)*exp(b))
  dev:   qkT = [Wq; Wk] @ xT                           [2C, N] bf16 (partition = feature)
         v_aug = [x[b] @ Wv.T | 1]                     [N, H, D+1] bf16
         per head pair hp, q-tile qt (512 q):
           S^T[k, q] = kT.T @ qT           (PE bf16, even head rows 0-63 /
                                            odd head rows 64-127, concurrent)
           e = exp(0.125 * S^T)            (ACT, multi-bank PSUM read, bf16 out)
           P = e * emt                     (DVE bf16 2x)
           [O^T; l] += [v|1].T @ P         (PE bf16, PSUM accum over k-tiles)
           ao_h = O^T[0:64] * (1/l)        (l broadcast to 64 partitions via a
                                            K=1 matmul from partition 64, then
                                            reciprocal + multiply on DVE)
         y[tok, :] = sum_h ao_h.T @ pw_h + b            (PE, 6 K=64 matmuls)

PSUM: pool A [128,2048] (4 banks, 2 kt-pairs) and pool B [128,1024] (2 banks,
1 kt-pair) alternate so the ACT exp of one group overlaps the S matmuls of the
next; pvp (2 banks) holds the two PV accumulators.
"""

from contextlib import ExitStack

import numpy as np
import ml_dtypes

import concourse.bass as bass
import concourse.mybir as mybir
from concourse import bacc
from concourse.tile import TileContext
from concourse.bass_utils import run_bass_kernel_spmd

F32 = mybir.dt.float32
BF16 = mybir.dt.bfloat16

B, N, C, H = 8, 2048, 384, 6
D = C // H          # 64
QT = N // 512       # 4  q-tiles of 512
KT = N // 128       # 16 k-tiles of 128
NT = N // 128       # 16 token tiles

# per (hp, qt): alternating PSUM-group pattern covering kt 0..15
# 'A' consumes 2 kt (4 banks: e,e,o,o), 'B' consumes 1 kt (2 banks: e,o)
GROUPS = [("A", (0, 1)), ("B", (2,)), ("A", (3, 4)), ("B", (5,)),
          ("A", (6, 7)), ("B", (8,)), ("A", (9, 10)), ("B", (11,)),
          ("A", (12, 13)), ("B", (14,)), ("B", (15,))]

# set by test harness to capture timing
TRACE = False
LAST_RESULT = None

_NC_CACHE = None


def build_nc():
    nc = bacc.Bacc("TRN2", target_bir_lowering=False, debug=False)

    xT = nc.declare_dram_parameter("xT", [C, N], BF16, isOutput=False)
    wqkT = nc.declare_dram_parameter("wqkT", [C, 2 * C], BF16, isOutput=False)
    wvT = nc.declare_dram_parameter("wvT", [C, C], BF16, isOutput=False)
    pwT = nc.declare_dram_parameter("pwT", [C, C], BF16, isOutput=False)
    pb = nc.declare_dram_parameter("pb", [1, C], F32, isOutput=False)
    emt = nc.declare_dram_parameter("emt", [N, N], BF16, isOutput=False)
    out = nc.declare_dram_parameter("out", [N, C], F32, isOutput=True)

    emt_r = emt.ap().rearrange("(t p) q -> p t q", p=128)  # [128, KT, N]

    with TileContext(nc) as tc:
        with ExitStack() as ctx:
            consts = ctx.enter_context(tc.tile_pool(name="consts", bufs=1))
            wpool = ctx.enter_context(tc.tile_pool(name="weights", bufs=1))
            qkpool = ctx.enter_context(tc.tile_pool(name="qk", bufs=1))
            vpool = ctx.enter_context(tc.tile_pool(name="v", bufs=1))
            psA = ctx.enter_context(tc.tile_pool(name="psA", bufs=1, space="PSUM"))
            psB = ctx.enter_context(tc.tile_pool(name="psB", bufs=1, space="PSUM"))
            pvp = ctx.enter_context(tc.tile_pool(name="pvp", bufs=2, space="PSUM"))

            # ---- constants ----
            ones_row = consts.tile([128, 64], BF16, tag="ones_row")
            nc.vector.memset(ones_row[:, :], 1.0)
            ones1 = consts.tile([1, 128], F32, tag="ones1")
            nc.vector.memset(ones1[:, :], 1.0)
            pb_sb = consts.tile([1, C], F32, tag="pbsb")
            nc.sync.dma_start(out=pb_sb[:, :], in_=pb[:, :])

            # ---- load x^T and weights ----
            xt_ctx = ExitStack()
            xtpool = xt_ctx.enter_context(tc.tile_pool(name="xt", bufs=1))
            xT_sb = []
            for i in range(3):
                t = xtpool.tile([128, N], BF16, tag=f"xT{i}")
                nc.sync.dma_start(out=t[:, :], in_=xT[i * 128:(i + 1) * 128, :])
                xT_sb.append(t)
            wqkT_sb = []
            for i in range(3):
                t = wpool.tile([128, 2 * C], BF16, tag=f"wqk{i}")
                nc.sync.dma_start(out=t[:, :], in_=wqkT[i * 128:(i + 1) * 128, :])
                wqkT_sb.append(t)
            wvT_sb = []
            for i in range(3):
                t = wpool.tile([128, C], BF16, tag=f"wv{i}")
                nc.sync.dma_start(out=t[:, :], in_=wvT[i * 128:(i + 1) * 128, :])
                wvT_sb.append(t)
            # proj weights, one [64, C] tile per head (all at partition base 0)
            pw6_sb = []
            for h in range(H):
                t = wpool.tile([64, C], BF16, tag=f"pw{h}")
                nc.sync.dma_start(out=t[:, :], in_=pwT[h * 64:(h + 1) * 64, :])
                pw6_sb.append(t)

            # ---- phase B: qkT = [Wq; Wk] @ xT  -> [768, N] (6 partition tiles) ----
            qkT_sb = []
            for f in range(6):
                qkT_sb.append(qkpool.tile([128, N], BF16, name=f"qk{f}", tag=f"qk{f}"))
            for f in range(6):
                for j in range(QT):
                    ps = pvp.tile([128, 512], F32, tag="pv")
                    for kc in range(3):
                        nc.tensor.matmul(
                            ps[:, :],
                            wqkT_sb[kc][:, f * 128:(f + 1) * 128],
                            xT_sb[kc][:, j * 512:(j + 1) * 512],
                            start=(kc == 0),
                            stop=(kc == 2),
                        )
                    nc.scalar.copy(qkT_sb[f][:, j * 512:(j + 1) * 512], ps[:, :])

            # ---- phase C: v_aug [N, H, D+1] bf16, ones in last column ----
            vaug_sb = []
            for t_i in range(NT):
                vaug_sb.append(
                    vpool.tile([128, H, D + 1], BF16, name=f"va{t_i}", tag=f"va{t_i}")
                )
            for t_i in range(NT):
                ps = pvp.tile([128, C], F32, tag="pv")
                for kc in range(3):
                    nc.tensor.matmul(
                        ps[:, :],
                        xT_sb[kc][:, t_i * 128:(t_i + 1) * 128],
                        wvT_sb[kc][:, :],
                        start=(kc == 0),
                        stop=(kc == 2),
                    )
                nc.vector.memset(vaug_sb[t_i][:, :, :], 1.0)
                nc.scalar.copy(
                    vaug_sb[t_i][:, :, 0:D],
                    ps[:, :].rearrange("p (h d) -> p h d", d=D),
                )

            # xT no longer needed; free its SBUF for the attention pools
            xt_ctx.close()
            emtp = ctx.enter_context(tc.tile_pool(name="emtp", bufs=3))
            epool = ctx.enter_context(tc.tile_pool(name="e", bufs=2))
            ppool = ctx.enter_context(tc.tile_pool(name="p", bufs=3))
            aopool = ctx.enter_context(tc.tile_pool(name="ao", bufs=12))
            lpool = ctx.enter_context(tc.tile_pool(name="l", bufs=2))
            ypool = ctx.enter_context(tc.tile_pool(name="y", bufs=2))

            def s_mm(sp_slice, hp, head_off, kt, qt):
                """One S^T matmul: [64,128] kT (stationary) x [64,512] qT."""
                nc.tensor.matmul(
                    sp_slice,
                    qkT_sb[3 + hp][head_off:head_off + 64, kt * 128:(kt + 1) * 128],
                    qkT_sb[hp][head_off:head_off + 64, qt * 512:(qt + 1) * 512],
                    start=True,
                    stop=True,
                )

            def pv_mm(pv_ps, hp, par, kt, pt_slice):
                nc.tensor.matmul(
                    pv_ps[0:D + 1, :],
                    vaug_sb[kt][:, 2 * hp + par, :],
                    pt_slice,
                    start=(kt == 0),
                    stop=(kt == KT - 1),
                )

            # ---- phase D: attention per q-tile ----
            for qt in range(QT):
                em_tiles = []
                for kg in range(2):
                    em = emtp.tile([128, 8 * 512], BF16, tag="emt")
                    nc.sync.dma_start(
                        out=em[:, :],
                        in_=emt_r[:, kg * 8:(kg + 1) * 8, qt * 512:(qt + 1) * 512],
                    )
                    em_tiles.append(em)

                ao_tiles = [None] * H
                for hp in range(3):
                    pv_e = pvp.tile([128, 512], F32, tag="pv")
                    pv_o = pvp.tile([128, 512], F32, tag="pv")
                    for gkind, kts in GROUPS:
                        if gkind == "A":
                            kta, ktb = kts
                            sp = psA.tile([128, 2048], F32, tag="sA")
                            # [e_kta | e_ktb | o_kta | o_ktb]; issue order
                            # alternates row groups so pairs run concurrently
                            s_mm(sp[:, 0:512], hp, 0, kta, qt)
                            s_mm(sp[:, 1024:1536], hp, 64, kta, qt)
                            s_mm(sp[:, 512:1024], hp, 0, ktb, qt)
                            s_mm(sp[:, 1536:2048], hp, 64, ktb, qt)
                            et = epool.tile([128, 2048], BF16, tag="eA")
                            nc.scalar.activation(
                                et[:, :], sp[:, :],
                                mybir.ActivationFunctionType.Exp, scale=0.125,
                            )
                            pt = ppool.tile([128, 2048], BF16, tag="pA")
                            emsl = em_tiles[kta // 8][
                                :, (kta % 8) * 512:((kta % 8) + 2) * 512
                            ]
                            nc.vector.tensor_mul(pt[:, 0:1024], et[:, 0:1024], emsl)
                            nc.vector.tensor_mul(pt[:, 1024:2048], et[:, 1024:2048], emsl)
                            pv_mm(pv_e, hp, 0, kta, pt[:, 0:512])
                            pv_mm(pv_o, hp, 1, kta, pt[:, 1024:1536])
                            pv_mm(pv_e, hp, 0, ktb, pt[:, 512:1024])
                            pv_mm(pv_o, hp, 1, ktb, pt[:, 1536:2048])
                        else:
                            (kt,) = kts
                            sp = psB.tile([128, 1024], F32, tag="sB")
                            s_mm(sp[:, 0:512], hp, 0, kt, qt)
                            s_mm(sp[:, 512:1024], hp, 64, kt, qt)
                            et = epool.tile([128, 1024], BF16, tag="eB")
                            nc.scalar.activation(
                                et[:, :], sp[:, :],
                                mybir.ActivationFunctionType.Exp, scale=0.125,
                            )
                            pt = ppool.tile([128, 1024], BF16, tag="pB")
                            emsl = em_tiles[kt // 8][:, (kt % 8) * 512:((kt % 8) + 1) * 512]
                            nc.vector.tensor_mul(pt[:, 0:512], et[:, 0:512], emsl)
                            nc.vector.tensor_mul(pt[:, 512:1024], et[:, 512:1024], emsl)
                            pv_mm(pv_e, hp, 0, kt, pt[:, 0:512])
                            pv_mm(pv_o, hp, 1, kt, pt[:, 512:1024])

                    # l lives on partition 64 (row D of the [65,512] PV result).
                    # Copy both heads' l to SBUF partition 64 (bf16), broadcast
                    # down to partitions 0-63 with a K=1 matmul, then
                    # reciprocal + multiply on DVE at partition base 0.
                    lrow = lpool.tile([65, 1024], BF16, tag="lrow")
                    nc.vector.tensor_copy(lrow[64:65, 0:512], pv_e[D:D + 1, :])
                    nc.vector.tensor_copy(lrow[64:65, 512:1024], pv_o[D:D + 1, :])
                    bc = psB.tile([128, 1024], F32, tag="sB")
                    nc.tensor.matmul(
                        bc[0:64, 0:512], ones_row[64:65, :], lrow[64:65, 0:512],
                        start=True, stop=True,
                    )
                    nc.tensor.matmul(
                        bc[0:64, 512:1024], ones_row[64:65, :], lrow[64:65, 512:1024],
                        start=True, stop=True,
                    )
                    bcl = lpool.tile([64, 1024], F32, tag="bcl")
                    nc.vector.tensor_copy(bcl[:, :], bc[0:64, :])
                    bcs = lpool.tile([64, 1024], F32, tag="bcs")
                    nc.vector.reciprocal_approx_fast(out=bcs[:, :], in_=bcl[:, :])
                    ao_e = aopool.tile([64, 512], BF16, tag="ao")
                    ao_o = aopool.tile([64, 512], BF16, tag="ao")
                    nc.vector.tensor_mul(ao_e[:, :], pv_e[0:64, :], bcs[:, 0:512])
                    nc.vector.tensor_mul(ao_o[:, :], pv_o[0:64, :], bcs[:, 512:1024])
                    ao_tiles[2 * hp] = ao_e
                    ao_tiles[2 * hp + 1] = ao_o

                # ---- phase E: y = sum_h ao_h.T @ pw_h + b for this q-tile ----
                for tt in range(4):
                    ps = pvp.tile([128, C], F32, tag="pv")
                    for h in range(H):
                        nc.tensor.matmul(
                            ps[:, :],
                            ao_tiles[h][:, tt * 128:(tt + 1) * 128],
                            pw6_sb[h][:, :],
                            start=(h == 0),
                            stop=False,
                        )
                    nc.tensor.matmul(
                        ps[:, :], ones1[:, :], pb_sb[:, :], start=False, stop=True
                    )
                    yt = ypool.tile([128, C], F32, tag="y")
                    nc.scalar.copy(yt[:, :], ps[:, :])
                    row = (qt * 4 + tt) * 128
                    nc.sync.dma_start(out=out[row:row + 128, :], in_=yt[:, :])

    nc.compile()
    return nc


def _get_nc():
    global _NC_CACHE
    if _NC_CACHE is None:
        _NC_CACHE = build_nc()
    return _NC_CACHE


def kernel(**inputs):
    x = np.asarray(inputs["x"], dtype=np.float32)
    mask = np.asarray(inputs["mask"], dtype=np.float32)
    qkv_w = np.asarray(inputs["qkv_w"], dtype=np.float32)
    proj_w = np.asarray(inputs["proj_w"], dtype=np.float32)
    proj_b = np.asarray(inputs["proj_b"], dtype=np.float32)

    nc = _get_nc()

    bf16 = ml_dtypes.bfloat16
    wqkT = np.ascontiguousarray(qkv_w[:2 * C].T.astype(bf16))
    wvT = np.ascontiguousarray(qkv_w[2 * C:].T.astype(bf16))
    pwT = np.ascontiguousarray(proj_w.T.astype(bf16))
    pb = np.ascontiguousarray(proj_b.reshape(1, C).astype(np.float32))

    in_maps = []
    for b in range(B):
        xTb = np.ascontiguousarray(x[b].T.astype(bf16))
        mm = mask[b] - mask[b].min(axis=1, keepdims=True)
        emtb = np.exp(-1e5 * mm).T.astype(bf16)
        in_maps.append(
            {
                "xT": xTb,
                "wqkT": wqkT,
                "wvT": wvT,
                "pwT": pwT,
                "pb": pb,
                "emt": np.ascontiguousarray(emtb),
            }
        )

    global LAST_RESULT
    res = run_bass_kernel_spmd(nc, in_maps, core_ids=list(range(B)), trace=TRACE)
    LAST_RESULT = res
    return np.stack([res.results[b]["out"] for b in range(B)]).astype(np.float32)


# revision 7
# speedup vs baseline: 3.7452x; 1.1397x over previous
"""Trainium2 Bass kernel for nn_Attention_3599182594919.

Multi-head attention, B=8 N=2048 C=384 H=6 D=64, data-parallel over batch
across 8 NeuronCores (one batch element per core, no collectives).

Per-core algorithm (layouts chosen so no on-chip transposes are needed and
every DVE op keeps all operands on the same partition window):
  host:  xT = x[b].T bf16                              [C, N]
         emt[k, q] = exp(-1e5*(mask[b][q,k] - min_k mask[b][q,:]))  bf16 [N, N]
           (softmax max-shift folded into the mask factor; exp(a+b)=exp(a)*exp(b))
  dev:   qkT = [Wq; Wk] @ xT                           [2C, N] bf16 (partition = feature)
         v_aug = [x[b] @ Wv.T | 1]                     [N, H, D+1] bf16
         per head pair hp, q-tile qt (512 q):
           S^T[k, q] = kT.T @ qT           (PE bf16, even head rows 0-63 /
                                            odd head rows 64-127, concurrent)
           e = exp(0.125 * S^T)            (ACT, multi-bank PSUM read, bf16 out)
           P = e * emt                     (DVE bf16 2x)
           [O^T; l] += [v|1].T @ P         (PE bf16, PSUM accum over k-tiles)
           ao_h = O^T[0:64] * (1/l)        (l broadcast to 64 partitions via a
                                            K=1 matmul from partition 64, then
                                            reciprocal + multiply on DVE)
         y[tok, :] = sum_h ao_h.T @ pw_h + b            (PE, 6 K=64 matmuls)

PSUM: pool A [128,2048] (4 banks, 2 kt-pairs) and pool B [128,1024] (2 banks,
1 kt-pair) alternate so the ACT exp of one group overlaps the S matmuls of the
next; pvp (2 banks) holds the two PV accumulators.
"""

from contextlib import ExitStack

import numpy as np
import ml_dtypes

import concourse.bass as bass
import concourse.mybir as mybir
from concourse import bacc
from concourse.tile import TileContext
from concourse.bass_utils import run_bass_kernel_spmd

F32 = mybir.dt.float32
BF16 = mybir.dt.bfloat16

B, N, C, H = 8, 2048, 384, 6
D = C // H          # 64
QT = N // 512       # 4  q-tiles of 512
KT = N // 128       # 16 k-tiles of 128
NT = N // 128       # 16 token tiles

# per (hp, qt): alternating PSUM-group pattern covering kt 0..15
# 'A' consumes 2 kt (4 banks: e,e,o,o), 'B' consumes 1 kt (2 banks: e,o)
GROUPS = [("A", (0, 1)), ("B", (2,)), ("A", (3, 4)), ("B", (5,)),
          ("A", (6, 7)), ("B", (8,)), ("A", (9, 10)), ("B", (11,)),
          ("A", (12, 13)), ("B", (14,)), ("B", (15,))]

# set by test harness to capture timing
TRACE = False
LAST_RESULT = None

_NC_CACHE = None


def build_nc():
    nc = bacc.Bacc("TRN2", target_bir_lowering=False, debug=False)

    xT = nc.declare_dram_parameter("xT", [C, N], BF16, isOutput=False)
    wqkT = nc.declare_dram_parameter("wqkT", [C, 2 * C], BF16, isOutput=False)
    wvT = nc.declare_dram_parameter("wvT", [C, C], BF16, isOutput=False)
    pwT = nc.declare_dram_parameter("pwT", [C, C], BF16, isOutput=False)
    pb = nc.declare_dram_parameter("pb", [1, C], F32, isOutput=False)
    emt = nc.declare_dram_parameter("emt", [N, N], BF16, isOutput=False)
    out = nc.declare_dram_parameter("out", [N, C], F32, isOutput=True)

    emt_r = emt.ap().rearrange("(t p) q -> p t q", p=128)  # [128, KT, N]

    with TileContext(nc) as tc:
        with ExitStack() as ctx:
            consts = ctx.enter_context(tc.tile_pool(name="consts", bufs=1))
            wpool = ctx.enter_context(tc.tile_pool(name="weights", bufs=1))
            qkpool = ctx.enter_context(tc.tile_pool(name="qk", bufs=1))
            vpool = ctx.enter_context(tc.tile_pool(name="v", bufs=1))
            psA = ctx.enter_context(tc.tile_pool(name="psA", bufs=1, space="PSUM"))
            psB = ctx.enter_context(tc.tile_pool(name="psB", bufs=1, space="PSUM"))
            pvp = ctx.enter_context(tc.tile_pool(name="pvp", bufs=2, space="PSUM"))

            # ---- constants ----
            ones_row = consts.tile([128, 64], BF16, tag="ones_row")
            nc.vector.memset(ones_row[:, :], 1.0)
            ones1 = consts.tile([1, 128], F32, tag="ones1")
            nc.vector.memset(ones1[:, :], 1.0)
            pb_sb = consts.tile([1, C], F32, tag="pbsb")
            nc.sync.dma_start(out=pb_sb[:, :], in_=pb[:, :])

            # ---- load x^T and weights ----
            xt_ctx = ExitStack()
            xtpool = xt_ctx.enter_context(tc.tile_pool(name="xt", bufs=1))
            xT_sb = []
            for i in range(3):
                t = xtpool.tile([128, N], BF16, tag=f"xT{i}")
                nc.sync.dma_start(out=t[:, :], in_=xT[i * 128:(i + 1) * 128, :])
                xT_sb.append(t)
            wqkT_sb = []
            for i in range(3):
                t = wpool.tile([128, 2 * C], BF16, tag=f"wqk{i}")
                nc.sync.dma_start(out=t[:, :], in_=wqkT[i * 128:(i + 1) * 128, :])
                wqkT_sb.append(t)
            wvT_sb = []
            for i in range(3):
                t = wpool.tile([128, C], BF16, tag=f"wv{i}")
                nc.sync.dma_start(out=t[:, :], in_=wvT[i * 128:(i + 1) * 128, :])
                wvT_sb.append(t)
            # proj weights, one [64, C] tile per head (all at partition base 0)
            pw6_sb = []
            for h in range(H):
                t = wpool.tile([64, C], BF16, tag=f"pw{h}")
                nc.sync.dma_start(out=t[:, :], in_=pwT[h * 64:(h + 1) * 64, :])
                pw6_sb.append(t)

            # ---- phase B: qkT = [Wq; Wk] @ xT  -> [768, N] (6 partition tiles) ----
            qkT_sb = []
            for f in range(6):
                qkT_sb.append(qkpool.tile([128, N], BF16, name=f"qk{f}", tag=f"qk{f}"))
            for f in range(6):
                for j in range(QT):
                    ps = pvp.tile([128, 512], F32, tag="pv")
                    for kc in range(3):
                        nc.tensor.matmul(
                            ps[:, :],
                            wqkT_sb[kc][:, f * 128:(f + 1) * 128],
                            xT_sb[kc][:, j * 512:(j + 1) * 512],
                            start=(kc == 0),
                            stop=(kc == 2),
                        )
                    nc.vector.tensor_copy(qkT_sb[f][:, j * 512:(j + 1) * 512], ps[:, :])

            # ---- phase C: v_aug [N, H, D+1] bf16, ones in last column ----
            vaug_sb = []
            for t_i in range(NT):
                vaug_sb.append(
                    vpool.tile([128, H, D + 1], BF16, name=f"va{t_i}", tag=f"va{t_i}")
                )
            for t_i in range(NT):
                ps = pvp.tile([128, C], F32, tag="pv")
                for kc in range(3):
                    nc.tensor.matmul(
                        ps[:, :],
                        xT_sb[kc][:, t_i * 128:(t_i + 1) * 128],
                        wvT_sb[kc][:, :],
                        start=(kc == 0),
                        stop=(kc == 2),
                    )
                nc.vector.memset(vaug_sb[t_i][:, :, :], 1.0)
                nc.vector.tensor_copy(
                    vaug_sb[t_i][:, :, 0:D],
                    ps[:, :].rearrange("p (h d) -> p h d", d=D),
                )

            # xT no longer needed; free its SBUF for the attention pools
            xt_ctx.close()
            emtp = ctx.enter_context(tc.tile_pool(name="emtp", bufs=3))
            epool = ctx.enter_context(tc.tile_pool(name="e", bufs=2))
            ppool = ctx.enter_context(tc.tile_pool(name="p", bufs=3))
            aopool = ctx.enter_context(tc.tile_pool(name="ao", bufs=12))
            lpool = ctx.enter_context(tc.tile_pool(name="l", bufs=2))
            ypool = ctx.enter_context(tc.tile_pool(name="y", bufs=2))

            def s_mm(sp_slice, hp, head_off, kt, qt):
                """One S^T matmul: [64,128] kT (stationary) x [64,512] qT."""
                nc.tensor.matmul(
                    sp_slice,
                    qkT_sb[3 + hp][head_off:head_off + 64, kt * 128:(kt + 1) * 128],
                    qkT_sb[hp][head_off:head_off + 64, qt * 512:(qt + 1) * 512],
                    start=True,
                    stop=True,
                )

            def pv_mm(pv_ps, hp, par, kt, pt_slice):
                nc.tensor.matmul(
                    pv_ps[0:D + 1, :],
                    vaug_sb[kt][:, 2 * hp + par, :],
                    pt_slice,
                    start=(kt == 0),
                    stop=(kt == KT - 1),
                )

            # ---- phase D: attention per q-tile ----
            for qt in range(QT):
                em_tiles = []
                for kg in range(2):
                    em = emtp.tile([128, 8 * 512], BF16, tag="emt")
                    nc.sync.dma_start(
                        out=em[:, :],
                        in_=emt_r[:, kg * 8:(kg + 1) * 8, qt * 512:(qt + 1) * 512],
                    )
                    em_tiles.append(em)

                pvu_tiles = [None] * H
                for hp in range(3):
                    pv_e = pvp.tile([128, 512], F32, tag="pv")
                    pv_o = pvp.tile([128, 512], F32, tag="pv")
                    for gkind, kts in GROUPS:
                        if gkind == "A":
                            kta, ktb = kts
                            sp = psA.tile([128, 2048], F32, tag="sA")
                            # [e_kta | e_ktb | o_kta | o_ktb]; issue order
                            # alternates row groups so pairs run concurrently
                            s_mm(sp[:, 0:512], hp, 0, kta, qt)
                            s_mm(sp[:, 1024:1536], hp, 64, kta, qt)
                            s_mm(sp[:, 512:1024], hp, 0, ktb, qt)
                            s_mm(sp[:, 1536:2048], hp, 64, ktb, qt)
                            et = epool.tile([128, 2048], BF16, tag="eA")
                            nc.scalar.activation(
                                et[:, :], sp[:, :],
                                mybir.ActivationFunctionType.Exp, scale=0.125,
                            )
                            pt = ppool.tile([128, 2048], BF16, tag="pA")
                            emsl = em_tiles[kta // 8][
                                :, (kta % 8) * 512:((kta % 8) + 2) * 512
                            ]
                            nc.vector.tensor_mul(pt[:, 0:1024], et[:, 0:1024], emsl)
                            nc.vector.tensor_mul(pt[:, 1024:2048], et[:, 1024:2048], emsl)
                            pv_mm(pv_e, hp, 0, kta, pt[:, 0:512])
                            pv_mm(pv_o, hp, 1, kta, pt[:, 1024:1536])
                            pv_mm(pv_e, hp, 0, ktb, pt[:, 512:1024])
                            pv_mm(pv_o, hp, 1, ktb, pt[:, 1536:2048])
                        else:
                            (kt,) = kts
                            sp = psB.tile([128, 1024], F32, tag="sB")
                            s_mm(sp[:, 0:512], hp, 0, kt, qt)
                            s_mm(sp[:, 512:1024], hp, 64, kt, qt)
                            et = epool.tile([128, 1024], BF16, tag="eB")
                            nc.scalar.activation(
                                et[:, :], sp[:, :],
                                mybir.ActivationFunctionType.Exp, scale=0.125,
                            )
                            pt = ppool.tile([128, 1024], BF16, tag="pB")
                            emsl = em_tiles[kt // 8][:, (kt % 8) * 512:((kt % 8) + 1) * 512]
                            nc.vector.tensor_mul(pt[:, 0:512], et[:, 0:512], emsl)
                            nc.vector.tensor_mul(pt[:, 512:1024], et[:, 512:1024], emsl)
                            pv_mm(pv_e, hp, 0, kt, pt[:, 0:512])
                            pv_mm(pv_o, hp, 1, kt, pt[:, 512:1024])

                    # Evacuate [O^T; l] to SBUF immediately (bf16) so the
                    # PSUM accumulators free up for the next head pair.
                    pvu_e = aopool.tile([65, 512], BF16, tag="pvu", name=f"pvu{qt}_{hp}e")
                    pvu_o = aopool.tile([65, 512], BF16, tag="pvu", name=f"pvu{qt}_{hp}o")
                    nc.vector.tensor_copy(pvu_e[:, :], pv_e[0:D + 1, :])
                    nc.vector.tensor_copy(pvu_o[:, :], pv_o[0:D + 1, :])
                    pvu_tiles[2 * hp] = pvu_e
                    pvu_tiles[2 * hp + 1] = pvu_o

                # ---- batched normalization for all 6 heads of this q-tile ----
                # broadcast l from partition 64 down to partitions 0-63 with
                # K=1 matmuls, reciprocal, then multiply.
                bcA = psA.tile([128, 2048], F32, tag="sA")
                for h in range(4):
                    nc.tensor.matmul(
                        bcA[0:64, h * 512:(h + 1) * 512],
                        ones_row[64:65, :], pvu_tiles[h][64:65, :],
                        start=True, stop=True,
                    )
                bcB = psB.tile([128, 1024], F32, tag="sB")
                for h in range(2):
                    nc.tensor.matmul(
                        bcB[0:64, h * 512:(h + 1) * 512],
                        ones_row[64:65, :], pvu_tiles[4 + h][64:65, :],
                        start=True, stop=True,
                    )
                bcsA = lpool.tile([64, 2048], F32, tag="bcsA")
                bcsB = lpool.tile([64, 1024], F32, tag="bcsB")
                nc.vector.reciprocal_approx_fast(out=bcsA[:, :], in_=bcA[0:64, :])
                nc.vector.reciprocal_approx_fast(out=bcsB[:, :], in_=bcB[0:64, :])
                ao_tiles = []
                for h in range(H):
                    ao = aopool.tile([64, 512], BF16, tag="ao", name=f"ao{qt}_{h}")
                    bcs_sl = (bcsA[:, h * 512:(h + 1) * 512] if h < 4
                              else bcsB[:, (h - 4) * 512:(h - 3) * 512])
                    nc.vector.tensor_mul(ao[:, :], pvu_tiles[h][0:64, :], bcs_sl)
                    ao_tiles.append(ao)

                # ---- phase E: y = sum_h ao_h.T @ pw_h + b for this q-tile ----
                for tt in range(4):
                    ps = pvp.tile([128, C], F32, tag="pv")
                    for h in range(H):
                        nc.tensor.matmul(
                            ps[:, :],
                            ao_tiles[h][:, tt * 128:(tt + 1) * 128],
                            pw6_sb[h][:, :],
                            start=(h == 0),
                            stop=False,
                        )
                    nc.tensor.matmul(
                        ps[:, :], ones1[:, :], pb_sb[:, :], start=False, stop=True
                    )
                    yt = ypool.tile([128, C], F32, tag="y")
                    nc.vector.tensor_copy(yt[:, :], ps[:, :])
                    row = (qt * 4 + tt) * 128
                    nc.sync.dma_start(out=out[row:row + 128, :], in_=yt[:, :])

    nc.compile()
    return nc


def _get_nc():
    global _NC_CACHE
    if _NC_CACHE is None:
        _NC_CACHE = build_nc()
    return _NC_CACHE


def kernel(**inputs):
    x = np.asarray(inputs["x"], dtype=np.float32)
    mask = np.asarray(inputs["mask"], dtype=np.float32)
    qkv_w = np.asarray(inputs["qkv_w"], dtype=np.float32)
    proj_w = np.asarray(inputs["proj_w"], dtype=np.float32)
    proj_b = np.asarray(inputs["proj_b"], dtype=np.float32)

    nc = _get_nc()

    bf16 = ml_dtypes.bfloat16
    wqkT = np.ascontiguousarray(qkv_w[:2 * C].T.astype(bf16))
    wvT = np.ascontiguousarray(qkv_w[2 * C:].T.astype(bf16))
    pwT = np.ascontiguousarray(proj_w.T.astype(bf16))
    pb = np.ascontiguousarray(proj_b.reshape(1, C).astype(np.float32))

    in_maps = []
    for b in range(B):
        xTb = np.ascontiguousarray(x[b].T.astype(bf16))
        mm = mask[b] - mask[b].min(axis=1, keepdims=True)
        emtb = np.exp(-1e5 * mm).T.astype(bf16)
        in_maps.append(
            {
                "xT": xTb,
                "wqkT": wqkT,
                "wvT": wvT,
                "pwT": pwT,
                "pb": pb,
                "emt": np.ascontiguousarray(emtb),
            }
        )

    global LAST_RESULT
    res = run_bass_kernel_spmd(nc, in_maps, core_ids=list(range(B)), trace=TRACE)
    LAST_RESULT = res
    return np.stack([res.results[b]["out"] for b in range(B)]).astype(np.float32)
